# revision 8
# baseline (speedup 1.0000x reference)
"""GNN message-passing (ADMETPredictor) distributed Bass kernel for 8 TRN2
NeuronCores.

Strategy (node sharding):
  - 200k nodes sharded 25k/core (padded to 25088 = 196*128 rows).
  - Edges grouped by owning (destination) core; within a core, grouped by
    source chunk (for int16-indexed dma_gather tables) and colored by
    per-destination occurrence so each dma_scatter_add call has no duplicate
    destination rows (HW CCE RMW races on duplicates).
  - h is exchanged per layer via AllGather (bf16); mean-neighbor aggregation is
    dma_gather (random 512B rows from the gathered table) + dma_scatter_add
    into a local accumulator.
  - Dense path: u = h + rdeg*agg -> PE transpose -> bf16 matmul (psum f32)
    -> batchnorm stats (free-dim accumulate + tiny AllReduce) -> scale/shift
    relu -> transpose back. Final layer accumulates the mean-pool instead.
  - Head (pool + task heads + sigmoid) computed redundantly on every core.
"""
import sys

sys.path.insert(0, '/opt/trn_rl_repo')

import numpy as np
import ml_dtypes

import concourse.bacc as bacc
import concourse.bass as bass
import concourse.tile as tile
import concourse.mybir as mybir
from concourse.bass_utils import run_bass_kernel_spmd
from concourse import masks
from concourse.library_config import mlp

F32 = mybir.dt.float32
BF16 = mybir.dt.bfloat16
I16 = mybir.dt.int16
AF = mybir.ActivationFunctionType
ALU = mybir.AluOpType

C = 8              # cores
H = 256            # hidden
F_IN = 148
L = 3
T_HEADS = 9
EPS = 1e-5
P = 128
SB_WIN = 1024      # gather window; SWDGE carveout caps ~1024 descs/instr


# ---------------------------------------------------------------- host prep

def _wrap_idx16(idx: np.ndarray) -> np.ndarray:
    """[n] -> [128, n//16] int16 wrapped layout (16-lane, replicated x8)."""
    n = idx.shape[0]
    assert n % 16 == 0
    w = idx.reshape(n // 16, 16).T.astype(np.int16)
    return np.tile(w, (8, 1))


def _round128(x: int) -> int:
    return (int(x) + 127) // 128 * 128


def prep_edges(edge_index: np.ndarray, n_nodes: int, npc: int, padn: int):
    """Bucket/color edges per core. Returns per-core idx arrays + the shared
    static schedule (segment caps, windows, scatter pieces)."""
    row = np.asarray(edge_index[0]).astype(np.int64)
    col = np.asarray(edge_index[1]).astype(np.int64)
    core = row // npc
    chunk = col // npc

    # per (core, chunk): local gather idx, local dest, color
    per_bucket = {}
    max_colors = np.zeros(C, np.int64)
    for c in range(C):
        for s in range(C):
            m = (core == c) & (chunk == s)
            d = row[m] - c * npc          # local dest
            g = col[m] - s * npc          # local table idx
            # color = occurrence index of dest within bucket
            order = np.argsort(d, kind='stable')
            d, g = d[order], g[order]
            colr = np.arange(d.size) - np.searchsorted(d, d)
            per_bucket[c, s] = (d, g, colr)
            if d.size:
                max_colors[s] = max(max_colors[s], colr.max() + 1)

    # segment caps shared across cores: cap[s][k]
    caps = []
    for s in range(C):
        ck = []
        for k in range(int(max_colors[s])):
            mx = max(int((per_bucket[c, s][2] == k).sum()) for c in range(C))
            ck.append(_round128(max(mx, 1)))
        caps.append(ck)

    # slot layout
    bucket_start = []
    seg_ranges = []   # [(s, k, a, b)]
    ptr = 0
    for s in range(C):
        bucket_start.append(ptr)
        for k, cap in enumerate(caps[s]):
            seg_ranges.append((s, k, ptr, ptr + cap))
            ptr += cap
    tot = ptr
    bucket_start.append(tot)

    # windows per bucket (gather calls)
    windows = []  # (s, w0, w1)
    for s in range(C):
        b0, b1 = bucket_start[s], bucket_start[s + 1]
        w = b0
        while w < b1:
            w1 = min(w + SB_WIN, b1)
            windows.append((s, w, w1))
            w = w1

    # scatter pieces: segment ∩ window
    pieces = []  # (widx, p0, p1)
    for (s, k, a, b) in seg_ranges:
        for wi, (ws, w0, w1) in enumerate(windows):
            if ws != s:
                continue
            p0, p1 = max(a, w0), min(b, w1)
            if p0 < p1:
                pieces.append((wi, p0, p1))

    # per-core idx arrays
    gidx_all, sidx_all = [], []
    pad_dest = padn - 1
    for c in range(C):
        gi = np.zeros(tot, np.int64)
        si = np.full(tot, pad_dest, np.int64)
        for (s, k, a, b) in seg_ranges:
            d, g, colr = per_bucket[c, s]
            m = colr == k
            n = int(m.sum())
            assert n <= b - a
            gi[a:a + n] = g[m]
            si[a:a + n] = d[m]
        gidx_all.append(_wrap_idx16(gi))
        sidx_all.append(_wrap_idx16(si))

    meta = dict(tot=tot, windows=windows, pieces=pieces)
    return gidx_all, sidx_all, meta


# ---------------------------------------------------------------- builder

def build(meta, npc, padn, n_nodes, dbg=False):
    """Build the SPMD Bass graph (same for all cores)."""
    G = padn // 512            # 512-node groups
    TILES = padn // 128
    LASTV = npc - (G - 1) * 512   # valid nodes in last group (<= 512)
    tot = meta['tot']
    windows = meta['windows']
    pieces = meta['pieces']
    WBUFN = max(w1 - w0 for (_, w0, w1) in windows)

    nc = bacc.Bacc("TRN2", target_bir_lowering=False, num_swdge_queues=4)

    # ---- parameters
    xT_e = nc.declare_dram_parameter("xT", [F_IN, padn], BF16, isOutput=False)
    rdeg_e = nc.declare_dram_parameter("rdeg", [padn], F32, isOutput=False)
    gidx_e = nc.declare_dram_parameter("gidx", [P, tot // 16], I16, isOutput=False)
    sidx_e = nc.declare_dram_parameter("sidx", [P, tot // 16], I16, isOutput=False)
    winT_e = nc.declare_dram_parameter("winT", [F_IN, H], BF16, isOutput=False)
    wg_e = nc.declare_dram_parameter("wgT", [L, H, H], BF16, isOutput=False)
    wp_e = nc.declare_dram_parameter("wpT", [H, H], BF16, isOutput=False)
    wh_e = nc.declare_dram_parameter("whT", [H, T_HEADS], BF16, isOutput=False)
    # coeffs f32 [128, ncol]: b_in(2) gamma(3*2) beta(3*2) b_pool(2) b_heads(1)
    NCOEF = 2 + L * 2 + L * 2 + 2 + 1
    coef_e = nc.declare_dram_parameter("coef", [P, NCOEF], F32, isOutput=False)
    out_e = nc.declare_dram_parameter("out", [T_HEADS], F32, isOutput=True)

    # ---- internal DRAM
    hin = [nc.dram_tensor(f"hin{l}", [padn, H], BF16) for l in range(L)]
    hfull = [nc.dram_tensor(f"hfull{l}", [C * padn, H], BF16,
                            addr_space="Shared") for l in range(L)]
    agg = [nc.dram_tensor(f"agg{l}", [padn, H], BF16) for l in range(L)]
    zt = [nc.dram_tensor(f"zt{l}", [H, padn], BF16) for l in range(L)]
    st_in = [nc.dram_tensor(f"stin{l}", [P, 4], F32) for l in range(L)]
    st_out = [nc.dram_tensor(f"stout{l}", [P, 4], F32, addr_space="Shared")
              for l in range(L)]
    g_in = nc.dram_tensor("g_in", [P, 2], F32)
    g_out = nc.dram_tensor("g_out", [P, 2], F32, addr_space="Shared")

    nc.gpsimd.load_library(mlp)

    rg = [list(range(C))]

    with tile.TileContext(nc) as tc:
        import contextlib
        with contextlib.ExitStack() as ctx:
            sing = ctx.enter_context(tc.tile_pool(name="sing", bufs=1))
            io = ctx.enter_context(tc.tile_pool(name="io", bufs=3))
            dio = ctx.enter_context(tc.tile_pool(name="dio", bufs=3))
            gpool = ctx.enter_context(tc.tile_pool(name="gpool", bufs=2))
            pz = ctx.enter_context(tc.tile_pool(name="pz", bufs=2, space="PSUM"))
            ptr = ctx.enter_context(tc.tile_pool(name="ptr", bufs=4, space="PSUM"))
            phead = ctx.enter_context(tc.tile_pool(name="phead", bufs=1, space="PSUM"))

            # ---- resident tiles
            ident = sing.tile([P, P], BF16)
            masks.make_identity(nc, ident[:])
            gidx_sb = sing.tile([P, tot // 16], I16)
            sidx_sb = sing.tile([P, tot // 16], I16)
            nc.sync.dma_start(out=gidx_sb[:], in_=gidx_e[:, :])
            nc.sync.dma_start(out=sidx_sb[:], in_=sidx_e[:, :])
            coef = sing.tile([P, NCOEF], F32)
            nc.sync.dma_start(out=coef[:], in_=coef_e[:, :])
            win_a = sing.tile([P, H], BF16)
            win_b = sing.tile([P, H], BF16)  # only first F_IN-128 rows used
            nc.sync.dma_start(out=win_a[:], in_=winT_e[0:P, :])
            nc.sync.dma_start(out=win_b[:F_IN - P, :], in_=winT_e[P:F_IN, :])
            wg_sb = sing.tile([P, L * 4, P], BF16)
            for l in range(L):
                for k in range(2):
                    for m in range(2):
                        nc.sync.dma_start(
                            out=wg_sb[:, l * 4 + k * 2 + m, :],
                            in_=wg_e[l, k * P:(k + 1) * P, m * P:(m + 1) * P])
            wp_sb = sing.tile([P, 4, P], BF16)
            for k in range(2):
                for m in range(2):
                    nc.sync.dma_start(out=wp_sb[:, k * 2 + m, :],
                                      in_=wp_e[k * P:(k + 1) * P, m * P:(m + 1) * P])
            wh_sb = sing.tile([P, 2, T_HEADS], BF16)
            for k in range(2):
                nc.sync.dma_start(out=wh_sb[:, k, :],
                                  in_=wh_e[k * P:(k + 1) * P, :])

            eps_sb = sing.tile([P, 1], F32)
            nc.vector.memset(eps_sb[:], float(EPS))
            b_in = [coef[:, m:m + 1] for m in range(2)]
            gam = [[coef[:, 2 + l * 2 + m:3 + l * 2 + m] for m in range(2)]
                   for l in range(L)]
            bet = [[coef[:, 2 + 2 * L + l * 2 + m:3 + 2 * L + l * 2 + m]
                    for m in range(2)] for l in range(L)]
            b_pool = [coef[:, 2 + 4 * L + m:3 + 4 * L + m] for m in range(2)]
            b_head = coef[0:T_HEADS, 2 + 4 * L + 2:3 + 4 * L + 2]

            # ============================================================
            # helper: transpose [128ch,512n] ch-major sbuf -> node-major dst
            def transpose_to_rm(src_sb, m, dst_sb):
                for t in range(4):
                    tp = ptr.tile([P, P], BF16, tag="trp")
                    nc.tensor.transpose(tp[:], src_sb[:, t * P:(t + 1) * P],
                                        ident[:])
                    nc.scalar.copy(dst_sb[:, t, m * P:(m + 1) * P], tp[:])

            # ============================================================
            # Stage 0: input projection -> hin[0]
            for g in range(G):
                c0 = g * 512
                xa = io.tile([P, 512], BF16, tag="xa")
                xb = io.tile([P, 512], BF16, tag="xb")
                nc.sync.dma_start(out=xa[:], in_=xT_e[0:P, c0:c0 + 512])
                nc.sync.dma_start(out=xb[:F_IN - P, :],
                                  in_=xT_e[P:F_IN, c0:c0 + 512])
                hrm = dio.tile([P, 4, H], BF16, tag="hrm")
                for m in range(2):
                    zp = pz.tile([P, 512], F32, tag="zp")
                    nc.tensor.matmul(zp[:], win_a[:, m * P:(m + 1) * P], xa[:],
                                     start=True, stop=False)
                    nc.tensor.matmul(zp[:], win_b[:F_IN - P, m * P:(m + 1) * P],
                                     xb[:F_IN - P, :], start=False, stop=True)
                    hsb = io.tile([P, 512], BF16, tag="hsb")
                    nc.scalar.activation(hsb[:], zp[:], AF.Relu, bias=b_in[m])
                    if g == G - 1 and LASTV < 512:
                        nc.vector.memset(hsb[:, LASTV:], 0.0)
                    transpose_to_rm(hsb, m, hrm)
                nc.sync.dma_start(
                    out=hin[0][c0:c0 + 512, :].rearrange("(t p) d -> p t d", p=P),
                    in_=hrm[:])

            # ============================================================
            # GNN layers
            for l in range(L):
                # ---- AllGather h
                nc.gpsimd.collective_compute(
                    "AllGather", ALU.bypass, replica_groups=rg,
                    ins=[hin[l].ap().opt()], outs=[hfull[l].ap().opt()])

                # ---- zero agg
                if l == 0:
                    zt16 = sing.tile([P, 2048], BF16, tag="zt16")
                    nc.vector.memset(zt16[:], 0.0)
                nrow = padn // P
                step = 8
                for a0 in range(0, nrow, step):
                    a1 = min(a0 + step, nrow)
                    nc.sync.dma_start(
                        out=agg[l].ap().rearrange("(a p) d -> p a d", p=P)
                        [:, a0:a1, :],
                        in_=zt16[:, :(a1 - a0) * H].rearrange(
                            "p (a d) -> p a d", d=H))

                # ---- gather windows + scatter pieces
                for wi, (s, w0, w1) in enumerate(windows):
                    n = w1 - w0
                    gb = gpool.tile([P, WBUFN // P, H], BF16, tag="gb")
                    nc.gpsimd.dma_gather(
                        gb[:, :n // P, :],
                        hfull[l][s * padn:(s + 1) * padn, :],
                        gidx_sb[:, w0 // 16:w1 // 16],
                        n, n, H, queue_num=wi % 3)
                    for (pwi, p0, p1) in pieces:
                        if pwi != wi:
                            continue
                        np_ = p1 - p0
                        nc.gpsimd.dma_scatter_add(
                            agg[l][:, :],
                            gb[:, (p0 - w0) // P:(p1 - w0) // P, :],
                            sidx_sb[:, p0 // 16:p1 // 16],
                            np_, np_, H, queue_num=3)

                # ---- dense A: u = h + rdeg*agg ; z = W u ; stats
                ssum = dio.tile([P, G, 2], F32, tag="ssum")
                ssq = dio.tile([P, G, 2], F32, tag="ssq")
                for g in range(G):
                    c0 = g * 512
                    at = io.tile([P, 4, H], BF16, tag="at")
                    ht = io.tile([P, 4, H], BF16, tag="ht")
                    rd = io.tile([P, 4], F32, tag="rd")
                    nc.sync.dma_start(
                        out=at[:], in_=agg[l][c0:c0 + 512, :].rearrange(
                            "(t p) d -> p t d", p=P))
                    nc.sync.dma_start(
                        out=ht[:], in_=hin[l][c0:c0 + 512, :].rearrange(
                            "(t p) d -> p t d", p=P))
                    nc.sync.dma_start(
                        out=rd[:], in_=rdeg_e[c0:c0 + 512].rearrange(
                            "(t p) -> p t", p=P))
                    ut = io.tile([P, 4, H], BF16, tag="ut")
                    for t in range(4):
                        nc.vector.tensor_scalar(ut[:, t, :], at[:, t, :],
                                                rd[:, t:t + 1], None, ALU.mult)
                    nc.vector.tensor_tensor(ut[:], ut[:], ht[:], ALU.add)
                    # transpose u -> uT [2][128ch, 512n]
                    uT = [io.tile([P, 512], BF16, tag=f"uT{k}", name=f"uT{k}")
                          for k in range(2)]
                    for t in range(4):
                        for k in range(2):
                            tp = ptr.tile([P, P], BF16, tag="trp")
                            nc.tensor.transpose(tp[:], ut[:, t, k * P:(k + 1) * P],
                                                ident[:])
                            nc.vector.tensor_copy(uT[k][:, t * P:(t + 1) * P],
                                                  tp[:])
                    for m in range(2):
                        zp = pz.tile([P, 512], F32, tag="zp")
                        for k in range(2):
                            nc.tensor.matmul(zp[:], wg_sb[:, l * 4 + k * 2 + m, :],
                                             uT[k][:], start=(k == 0),
                                             stop=(k == 1))
                        zsb = io.tile([P, 512], BF16, tag="zsb")
                        nc.scalar.activation(zsb[:], zp[:], AF.Identity,
                                             accum_out=ssum[:, g, m:m + 1])
                        sq = io.tile([P, 512], F32, tag="sq")
                        nc.scalar.activation(sq[:], zp[:], AF.Square,
                                             accum_out=ssq[:, g, m:m + 1])
                        nc.sync.dma_start(
                            out=zt[l][m * P:(m + 1) * P, c0:c0 + 512],
                            in_=zsb[:])

                # ---- stats allreduce + norm coefficients
                stl = dio.tile([P, 4], F32, tag="stl")
                nc.vector.tensor_reduce(stl[:, 0:1], ssum[:, :, 0],
                                        mybir.AxisListType.X, ALU.add)
                nc.vector.tensor_reduce(stl[:, 1:2], ssum[:, :, 1],
                                        mybir.AxisListType.X, ALU.add)
                nc.vector.tensor_reduce(stl[:, 2:3], ssq[:, :, 0],
                                        mybir.AxisListType.X, ALU.add)
                nc.vector.tensor_reduce(stl[:, 3:4], ssq[:, :, 1],
                                        mybir.AxisListType.X, ALU.add)
                nc.sync.dma_start(out=st_in[l][:, :], in_=stl[:])
                nc.gpsimd.collective_compute(
                    "AllReduce", ALU.add, replica_groups=rg,
                    ins=[st_in[l].ap().opt()], outs=[st_out[l].ap().opt()])
                stg = dio.tile([P, 4], F32, tag="stg")
                nc.sync.dma_start(out=stg[:], in_=st_out[l][:, :])
                mu = dio.tile([P, 2], F32, tag="mu")
                va = dio.tile([P, 2], F32, tag="va")
                aco = dio.tile([P, 2], F32, tag="aco")
                cco = dio.tile([P, 2], F32, tag="cco")
                inv_n = 1.0 / float(n_nodes)
                nc.vector.tensor_scalar(mu[:], stg[:, 0:2], inv_n, None, ALU.mult)
                nc.vector.tensor_scalar(va[:], stg[:, 2:4], inv_n, None, ALU.mult)
                mu2 = dio.tile([P, 2], F32, tag="mu2")
                nc.vector.tensor_tensor(mu2[:], mu[:], mu[:], ALU.mult)
                nc.vector.tensor_tensor(va[:], va[:], mu2[:], ALU.subtract)
                # rstd = 1/sqrt(va+eps)
                nc.scalar.activation(va[:], va[:], AF.Sqrt, bias=eps_sb[:, 0:1])
                nc.vector.reciprocal(va[:], va[:])
                for m in range(2):
                    nc.vector.tensor_tensor(aco[:, m:m + 1], gam[l][m],
                                            va[:, m:m + 1], ALU.mult)
                    nc.vector.tensor_tensor(cco[:, m:m + 1], mu[:, m:m + 1],
                                            aco[:, m:m + 1], ALU.mult)
                    nc.vector.tensor_tensor(cco[:, m:m + 1], bet[l][m],
                                            cco[:, m:m + 1], ALU.subtract)

                # ---- dense B: h = relu(a*z + c) [+ transpose or pool-accum]
                if l < L - 1:
                    for g in range(G):
                        c0 = g * 512
                        hrm = dio.tile([P, 4, H], BF16, tag="hrm")
                        for m in range(2):
                            zin = io.tile([P, 512], BF16, tag="zin")
                            nc.sync.dma_start(
                                out=zin[:],
                                in_=zt[l][m * P:(m + 1) * P, c0:c0 + 512])
                            hsb = io.tile([P, 512], BF16, tag="hsb")
                            nc.scalar.activation(hsb[:], zin[:], AF.Relu,
                                                 bias=cco[:, m:m + 1],
                                                 scale=aco[:, m:m + 1])
                            if g == G - 1 and LASTV < 512:
                                nc.vector.memset(hsb[:, LASTV:], 0.0)
                            transpose_to_rm(hsb, m, hrm)
                        nc.sync.dma_start(
                            out=hin[l + 1][c0:c0 + 512, :].rearrange(
                                "(t p) d -> p t d", p=P),
                            in_=hrm[:])
                else:
                    gsum = dio.tile([P, G, 2], F32, tag="gsum")
                    for g in range(G):
                        c0 = g * 512
                        nv = LASTV if g == G - 1 else 512
                        for m in range(2):
                            zin = io.tile([P, 512], BF16, tag="zin")
                            nc.sync.dma_start(
                                out=zin[:],
                                in_=zt[l][m * P:(m + 1) * P, c0:c0 + 512])
                            hsb = io.tile([P, 512], BF16, tag="hsb")
                            nc.scalar.activation(hsb[:, :nv], zin[:, :nv],
                                                 AF.Relu,
                                                 bias=cco[:, m:m + 1],
                                                 scale=aco[:, m:m + 1],
                                                 accum_out=gsum[:, g, m:m + 1])

            # ============================================================
            # mean pool allreduce + head
            gl = dio.tile([P, 2], F32, tag="gl")
            nc.vector.tensor_reduce(gl[:, 0:1], gsum[:, :, 0],
                                    mybir.AxisListType.X, ALU.add)
            nc.vector.tensor_reduce(gl[:, 1:2], gsum[:, :, 1],
                                    mybir.AxisListType.X, ALU.add)
            nc.sync.dma_start(out=g_in[:, :], in_=gl[:])
            nc.gpsimd.collective_compute(
                "AllReduce", ALU.add, replica_groups=rg,
                ins=[g_in.ap().opt()], outs=[g_out.ap().opt()])
            gg = dio.tile([P, 2], F32, tag="gg")
            nc.sync.dma_start(out=gg[:], in_=g_out[:, :])
            gbf = dio.tile([P, 2], BF16, tag="gbf")
            nc.vector.tensor_scalar(gbf[:], gg[:], 1.0 / float(n_nodes), None,
                                    ALU.mult)
            gp = dio.tile([P, 2], BF16, tag="gp")
            for m in range(2):
                pp = phead.tile([P, 1], F32, tag="pp")
                for k in range(2):
                    nc.tensor.matmul(pp[:], wp_sb[:, k * 2 + m, :],
                                     gbf[:, k:k + 1], start=(k == 0),
                                     stop=(k == 1))
                nc.scalar.activation(gp[:, m:m + 1], pp[:], AF.Relu,
                                     bias=b_pool[m])
            ph = phead.tile([T_HEADS, 1], F32, tag="ph")
            for k in range(2):
                nc.tensor.matmul(ph[:], wh_sb[:, k, :], gp[:, k:k + 1],
                                 start=(k == 0), stop=(k == 1))
            osb = dio.tile([T_HEADS, 1], F32, tag="osb")
            nc.scalar.activation(osb[:], ph[:], AF.Sigmoid,
                                 bias=b_head)
            nc.sync.dma_start(out=out_e[0:T_HEADS], in_=osb[:, 0])

    nc.compile()
    return nc


# ---------------------------------------------------------------- kernel

def _bf16(x):
    return np.asarray(x, np.float32).astype(ml_dtypes.bfloat16)


def build_in_maps(inputs, gidx_all, sidx_all, npc, padn):
    """Assemble per-core input maps from full inputs + prepped edge indices."""
    x = np.asarray(inputs["x"], np.float32)
    n_nodes = x.shape[0]
    ei = np.asarray(inputs["edge_index"])
    row = ei[0].astype(np.int64)
    b_in, W_in = inputs["b_in"], inputs["W_in"]
    W_gnn, gamma, beta = inputs["W_gnn"], inputs["gamma"], inputs["beta"]
    W_pool, b_pool = inputs["W_pool"], inputs["b_pool"]
    W_heads, b_heads = inputs["W_heads"], inputs["b_heads"]

    deg = np.bincount(row, minlength=n_nodes).astype(np.float32)
    rdeg = 1.0 / np.maximum(deg, 1.0)

    NCOEF = 2 + L * 2 + L * 2 + 2 + 1
    coef = np.zeros((P, NCOEF), np.float32)
    for m in range(2):
        coef[:, m] = b_in[m * P:(m + 1) * P]
        for l in range(L):
            coef[:, 2 + l * 2 + m] = gamma[l][m * P:(m + 1) * P]
            coef[:, 2 + 2 * L + l * 2 + m] = beta[l][m * P:(m + 1) * P]
        coef[:, 2 + 4 * L + m] = b_pool[m * P:(m + 1) * P]
    coef[:T_HEADS, 2 + 4 * L + 2] = b_heads

    winT = _bf16(np.asarray(W_in, np.float32).T)               # [F_IN, H]
    wgT = _bf16(np.asarray(W_gnn, np.float32).transpose(0, 2, 1))  # [L,H,H]
    wpT = _bf16(np.asarray(W_pool, np.float32).T)
    whT = _bf16(np.asarray(W_heads, np.float32).T)             # [H, T]

    in_maps = []
    for c in range(C):
        xs = np.zeros((F_IN, padn), ml_dtypes.bfloat16)
        xs[:, :npc] = _bf16(x[c * npc:(c + 1) * npc].T)
        rs = np.zeros(padn, np.float32)
        rs[:npc] = rdeg[c * npc:(c + 1) * npc]
        in_maps.append({
            "xT": xs, "rdeg": rs,
            "gidx": gidx_all[c], "sidx": sidx_all[c],
            "winT": winT, "wgT": wgT, "wpT": wpT, "whT": whT,
            "coef": coef,
        })
    return in_maps


def run_gnn(x, edge_index, W_in, b_in, W_gnn, b_gnn, gamma, beta,
            W_pool, b_pool, W_heads, b_heads, npc, trace=False):
    n_nodes = x.shape[0]
    assert n_nodes == npc * C
    padn = _round128(npc)
    if (padn // 512) * 512 != padn:
        padn = ((npc + 511) // 512) * 512

    ei = np.asarray(edge_index)
    gidx_all, sidx_all, meta = prep_edges(ei, n_nodes, npc, padn)
    inputs = dict(x=x, edge_index=edge_index, W_in=W_in, b_in=b_in,
                  W_gnn=W_gnn, gamma=gamma, beta=beta, W_pool=W_pool,
                  b_pool=b_pool, W_heads=W_heads, b_heads=b_heads)
    in_maps = build_in_maps(inputs, gidx_all, sidx_all, npc, padn)

    nc = build(meta, npc, padn, n_nodes)
    res = run_bass_kernel_spmd(nc, in_maps, core_ids=list(range(C)),
                               trace=trace)
    out = np.asarray(res.results[0]["out"], np.float32)
    return out, res


def kernel(**inputs) -> np.ndarray:
    out, _ = run_gnn(npc=25000, **inputs)
    return out


# revision 11
# speedup vs baseline: 6.7790x; 6.7790x over previous
"""GNN message-passing (ADMETPredictor) distributed Bass kernel for 8 TRN2
NeuronCores.

Strategy (node sharding):
  - 200k nodes sharded 25k/core (padded to 25088 = 196*128 rows).
  - Edges grouped by owning (destination) core; within a core, grouped by
    source chunk (for int16-indexed dma_gather tables) and colored by
    per-destination occurrence so each dma_scatter_add call has no duplicate
    destination rows (HW CCE RMW races on duplicates).
  - h is exchanged per layer via AllGather (bf16); mean-neighbor aggregation is
    dma_gather (random 512B rows from the gathered table) + dma_scatter_add
    into a local accumulator.
  - Dense path: u = h + rdeg*agg -> PE transpose -> bf16 matmul (psum f32)
    -> batchnorm stats (free-dim accumulate + tiny AllReduce) -> scale/shift
    relu -> transpose back. Final layer accumulates the mean-pool instead.
  - Head (pool + task heads + sigmoid) computed redundantly on every core.
"""
import sys

sys.path.insert(0, '/opt/trn_rl_repo')

import numpy as np
import ml_dtypes

import concourse.bacc as bacc
import concourse.bass as bass
import concourse.tile as tile
import concourse.mybir as mybir
from concourse.bass_utils import run_bass_kernel_spmd
from concourse import masks
from concourse.library_config import mlp

F32 = mybir.dt.float32
BF16 = mybir.dt.bfloat16
I16 = mybir.dt.int16
AF = mybir.ActivationFunctionType
ALU = mybir.AluOpType

C = 8              # cores
H = 256            # hidden
F_IN = 148
L = 3
T_HEADS = 9
EPS = 1e-5
P = 128
SB_WIN = 1024      # gather window; SWDGE carveout caps ~1024 descs/instr


# ---------------------------------------------------------------- host prep

def _wrap_idx16(idx: np.ndarray) -> np.ndarray:
    """[n] -> [128, n//16] int16 wrapped layout (16-lane, replicated x8)."""
    n = idx.shape[0]
    assert n % 16 == 0
    w = idx.reshape(n // 16, 16).T.astype(np.int16)
    return np.tile(w, (8, 1))


def _round128(x: int) -> int:
    return (int(x) + 127) // 128 * 128


def prep_edges(edge_index: np.ndarray, n_nodes: int, npc: int, padn: int):
    """Bucket/color edges per core. Returns per-core idx arrays + the shared
    static schedule (segment caps, windows, scatter pieces)."""
    row = np.asarray(edge_index[0]).astype(np.int64)
    col = np.asarray(edge_index[1]).astype(np.int64)
    core = row // npc
    chunk = col // npc

    # per (core, chunk): local gather idx, local dest, color
    per_bucket = {}
    max_colors = np.zeros(C, np.int64)
    for c in range(C):
        for s in range(C):
            m = (core == c) & (chunk == s)
            d = row[m] - c * npc          # local dest
            g = col[m] - s * npc          # local table idx
            # color = occurrence index of dest within bucket
            order = np.argsort(d, kind='stable')
            d, g = d[order], g[order]
            colr = np.arange(d.size) - np.searchsorted(d, d)
            per_bucket[c, s] = (d, g, colr)
            if d.size:
                max_colors[s] = max(max_colors[s], colr.max() + 1)

    # segment caps shared across cores: cap[s][k]
    caps = []
    for s in range(C):
        ck = []
        for k in range(int(max_colors[s])):
            mx = max(int((per_bucket[c, s][2] == k).sum()) for c in range(C))
            ck.append(_round128(max(mx, 1)))
        caps.append(ck)

    # slot layout
    bucket_start = []
    seg_ranges = []   # [(s, k, a, b)]
    ptr = 0
    for s in range(C):
        bucket_start.append(ptr)
        for k, cap in enumerate(caps[s]):
            seg_ranges.append((s, k, ptr, ptr + cap))
            ptr += cap
    tot = ptr
    bucket_start.append(tot)

    # windows per bucket (gather calls)
    windows = []  # (s, w0, w1)
    for s in range(C):
        b0, b1 = bucket_start[s], bucket_start[s + 1]
        w = b0
        while w < b1:
            w1 = min(w + SB_WIN, b1)
            windows.append((s, w, w1))
            w = w1

    # scatter pieces: segment ∩ window
    pieces = []  # (widx, p0, p1)
    for (s, k, a, b) in seg_ranges:
        for wi, (ws, w0, w1) in enumerate(windows):
            if ws != s:
                continue
            p0, p1 = max(a, w0), min(b, w1)
            if p0 < p1:
                pieces.append((wi, p0, p1))

    # per-core idx arrays: per window, gather idxs then scatter idxs
    # (both 16-wrapped), concatenated -> "gsidx" [128, 2*tot/16]
    gidx_all, sidx_all = [], []
    pad_dest = padn - 1
    for c in range(C):
        gi = np.zeros(tot, np.int64)
        si = np.full(tot, pad_dest, np.int64)
        for (s, k, a, b) in seg_ranges:
            d, g, colr = per_bucket[c, s]
            m = colr == k
            n = int(m.sum())
            assert n <= b - a
            gi[a:a + n] = g[m]
            si[a:a + n] = d[m]
        gw = _wrap_idx16(gi)
        sw = _wrap_idx16(si)
        parts = []
        for (s, w0, w1) in windows:
            parts.append(gw[:, w0 // 16:w1 // 16])
            parts.append(sw[:, w0 // 16:w1 // 16])
        gidx_all.append(np.concatenate(parts, axis=1))
        sidx_all.append(None)

    meta = dict(tot=tot, windows=windows, pieces=pieces)
    return gidx_all, sidx_all, meta


# ---------------------------------------------------------------- builder

def build(meta, npc, padn, n_nodes, dbg=False):
    """Build the SPMD Bass graph (same for all cores)."""
    G = padn // 512            # 512-node groups
    TILES = padn // 128
    LASTV = npc - (G - 1) * 512   # valid nodes in last group (<= 512)
    tot = meta['tot']
    windows = meta['windows']
    pieces = meta['pieces']
    WBUFN = max(w1 - w0 for (_, w0, w1) in windows)

    nc = bacc.Bacc("TRN2", target_bir_lowering=False, num_swdge_queues=4)

    # ---- parameters
    xT_e = nc.declare_dram_parameter("xT", [F_IN, padn], BF16, isOutput=False)
    rdeg_e = nc.declare_dram_parameter("rdeg", [padn], F32, isOutput=False)
    gsidx_e = nc.declare_dram_parameter("gsidx", [P, 2 * tot // 16], I16,
                                        isOutput=False)
    winT_e = nc.declare_dram_parameter("winT", [F_IN, H], BF16, isOutput=False)
    wg_e = nc.declare_dram_parameter("wgT", [L, H, H], BF16, isOutput=False)
    wp_e = nc.declare_dram_parameter("wpT", [H, H], BF16, isOutput=False)
    wh_e = nc.declare_dram_parameter("whT", [H, T_HEADS], BF16, isOutput=False)
    # coeffs f32 [128, ncol]: b_in(2) gamma(3*2) beta(3*2) b_pool(2) b_heads(1)
    NCOEF = 2 + L * 2 + L * 2 + 2 + 1
    coef_e = nc.declare_dram_parameter("coef", [P, NCOEF], F32, isOutput=False)
    out_e = nc.declare_dram_parameter("out", [T_HEADS], F32, isOutput=True)

    # ---- internal DRAM
    hin = [nc.dram_tensor(f"hin{l}", [padn, H], BF16) for l in range(L)]
    hfull = [nc.dram_tensor(f"hfull{l}", [C * padn, H], BF16,
                            addr_space="Shared") for l in range(L)]
    agg = [nc.dram_tensor(f"agg{l}", [padn, H], BF16) for l in range(L)]
    zt = [nc.dram_tensor(f"zt{l}", [H, padn], BF16) for l in range(L)]
    st_in = [nc.dram_tensor(f"stin{l}", [P, 4], F32) for l in range(L)]
    st_out = [nc.dram_tensor(f"stout{l}", [P, 4], F32, addr_space="Shared")
              for l in range(L)]
    g_in = nc.dram_tensor("g_in", [P, 2], F32)
    g_out = nc.dram_tensor("g_out", [P, 2], F32, addr_space="Shared")

    nc.gpsimd.load_library(mlp)

    rg = [list(range(C))]

    with tile.TileContext(nc) as tc:
        import contextlib
        with contextlib.ExitStack() as ctx:
            sing = ctx.enter_context(tc.tile_pool(name="sing", bufs=1))
            io = ctx.enter_context(tc.tile_pool(name="io", bufs=3))
            dio = ctx.enter_context(tc.tile_pool(name="dio", bufs=3))
            gpool = ctx.enter_context(tc.tile_pool(name="gpool", bufs=2))
            pz = ctx.enter_context(tc.tile_pool(name="pz", bufs=2, space="PSUM"))
            ptr = ctx.enter_context(tc.tile_pool(name="ptr", bufs=4, space="PSUM"))
            phead = ctx.enter_context(tc.tile_pool(name="phead", bufs=1, space="PSUM"))

            # ---- resident tiles
            ident = sing.tile([P, P], BF16)
            masks.make_identity(nc, ident[:])
            rdeg_sb = sing.tile([P, padn // P], F32)
            nc.sync.dma_start(
                out=rdeg_sb[:], in_=rdeg_e[:].rearrange("(t p) -> p t", p=P))
            coef = sing.tile([P, NCOEF], F32)
            nc.sync.dma_start(out=coef[:], in_=coef_e[:, :])
            win_a = sing.tile([P, H], BF16)
            win_b = sing.tile([P, H], BF16)  # only first F_IN-128 rows used
            nc.sync.dma_start(out=win_a[:], in_=winT_e[0:P, :])
            nc.sync.dma_start(out=win_b[:F_IN - P, :], in_=winT_e[P:F_IN, :])
            wg_sb = sing.tile([P, L * 4, P], BF16)
            for l in range(L):
                for k in range(2):
                    for m in range(2):
                        nc.sync.dma_start(
                            out=wg_sb[:, l * 4 + k * 2 + m, :],
                            in_=wg_e[l, k * P:(k + 1) * P, m * P:(m + 1) * P])
            wp_sb = sing.tile([P, 4, P], BF16)
            for k in range(2):
                for m in range(2):
                    nc.sync.dma_start(out=wp_sb[:, k * 2 + m, :],
                                      in_=wp_e[k * P:(k + 1) * P, m * P:(m + 1) * P])
            wh_sb = sing.tile([P, 2, T_HEADS], BF16)
            for k in range(2):
                nc.sync.dma_start(out=wh_sb[:, k, :],
                                  in_=wh_e[k * P:(k + 1) * P, :])

            eps_sb = sing.tile([P, 1], F32)
            nc.vector.memset(eps_sb[:], float(EPS))
            b_in = [coef[:, m:m + 1] for m in range(2)]
            gam = [[coef[:, 2 + l * 2 + m:3 + l * 2 + m] for m in range(2)]
                   for l in range(L)]
            bet = [[coef[:, 2 + 2 * L + l * 2 + m:3 + 2 * L + l * 2 + m]
                    for m in range(2)] for l in range(L)]
            b_pool = [coef[:, 2 + 4 * L + m:3 + 4 * L + m] for m in range(2)]
            b_head = coef[0:T_HEADS, 2 + 4 * L + 2:3 + 4 * L + 2]

            # ============================================================
            # helper: transpose [128ch,512n] ch-major sbuf -> node-major dst
            def transpose_to_rm(src_sb, m, dst_sb):
                for t in range(4):
                    tp = ptr.tile([P, P], BF16, tag="trp")
                    nc.tensor.transpose(tp[:], src_sb[:, t * P:(t + 1) * P],
                                        ident[:])
                    nc.scalar.copy(dst_sb[:, t, m * P:(m + 1) * P], tp[:])

            # ============================================================
            # Stage 0: input projection -> hin[0]
            for g in range(G):
                c0 = g * 512
                xa = io.tile([P, 512], BF16, tag="xa")
                xb = io.tile([P, 512], BF16, tag="xb")
                nc.sync.dma_start(out=xa[:], in_=xT_e[0:P, c0:c0 + 512])
                nc.sync.dma_start(out=xb[:F_IN - P, :],
                                  in_=xT_e[P:F_IN, c0:c0 + 512])
                hrm = dio.tile([P, 4, H], BF16, tag="hrm")
                for m in range(2):
                    zp = pz.tile([P, 512], F32, tag="zp")
                    nc.tensor.matmul(zp[:], win_a[:, m * P:(m + 1) * P], xa[:],
                                     start=True, stop=False)
                    nc.tensor.matmul(zp[:], win_b[:F_IN - P, m * P:(m + 1) * P],
                                     xb[:F_IN - P, :], start=False, stop=True)
                    hsb = io.tile([P, 512], BF16, tag="hsb")
                    nc.scalar.activation(hsb[:], zp[:], AF.Relu, bias=b_in[m])
                    if g == G - 1 and LASTV < 512:
                        nc.vector.memset(hsb[:, LASTV:], 0.0)
                    transpose_to_rm(hsb, m, hrm)
                nc.sync.dma_start(
                    out=hin[0][c0:c0 + 512, :].rearrange("(t p) d -> p t d", p=P),
                    in_=hrm[:])

            # ============================================================
            # GNN layers
            for l in range(L):
                # ---- AllGather h
                nc.gpsimd.collective_compute(
                    "AllGather", ALU.bypass, replica_groups=rg,
                    ins=[hin[l].ap().opt()], outs=[hfull[l].ap().opt()])

                # ---- zero agg (DRAM) via broadcast DMAs from a zero tile
                if l == 0:
                    zt16 = sing.tile([P, 2048], BF16, tag="zt16")
                    nc.vector.memset(zt16[:], 0.0)
                nrow = padn // P
                for a0 in range(0, nrow, 8):
                    a1 = min(a0 + 8, nrow)
                    nc.sync.dma_start(
                        out=agg[l].ap().rearrange("(a p) d -> p a d", p=P)
                        [:, a0:a1, :],
                        in_=zt16[:, :(a1 - a0) * H].rearrange(
                            "p (a d) -> p a d", d=H))

                # ---- gather windows + scatter pieces
                gcol = 0
                for wi, (s, w0, w1) in enumerate(windows):
                    n = w1 - w0
                    idxw = io.tile([P, 2 * WBUFN // 16], I16, tag="idxw")
                    nc.sync.dma_start(
                        out=idxw[:, :2 * n // 16],
                        in_=gsidx_e[:, gcol:gcol + 2 * n // 16])
                    gb = gpool.tile([P, WBUFN // P, H], BF16, tag="gb")
                    nc.gpsimd.dma_gather(
                        gb[:, :n // P, :],
                        hfull[l][s * padn:(s + 1) * padn, :],
                        idxw[:, :n // 16],
                        n, n, H, queue_num=wi % 3)
                    for (pwi, p0, p1) in pieces:
                        if pwi != wi:
                            continue
                        np_ = p1 - p0
                        nc.gpsimd.dma_scatter_add(
                            agg[l][:, :], gb[:, (p0 - w0) // P:(p1 - w0) // P, :],
                            idxw[:, (n + p0 - w0) // 16:(n + p1 - w0) // 16],
                            np_, np_, H, queue_num=3)
                    gcol += 2 * n // 16

                # ---- dense A: u = h + rdeg*agg ; z = W u ; stats
                ssum = dio.tile([P, G, 2], F32, tag="ssum")
                ssq = dio.tile([P, G, 2], F32, tag="ssq")
                for g in range(G):
                    c0 = g * 512
                    ht = io.tile([P, 4, H], BF16, tag="ht")
                    nc.sync.dma_start(
                        out=ht[:], in_=hin[l][c0:c0 + 512, :].rearrange(
                            "(t p) d -> p t d", p=P))
                    at = io.tile([P, 4, H], BF16, tag="at")
                    nc.sync.dma_start(
                        out=at[:], in_=agg[l][c0:c0 + 512, :].rearrange(
                            "(t p) d -> p t d", p=P))
                    ut = io.tile([P, 4, H], BF16, tag="ut")
                    for t in range(4):
                        nc.vector.tensor_scalar(
                            ut[:, t, :], at[:, t, :],
                            rdeg_sb[:, 4 * g + t:4 * g + t + 1], None, ALU.mult)
                    nc.vector.tensor_tensor(ut[:], ut[:], ht[:], ALU.add)
                    # transpose u -> uT [2][128ch, 512n]
                    uT = [io.tile([P, 512], BF16, tag=f"uT{k}", name=f"uT{k}")
                          for k in range(2)]
                    for t in range(4):
                        for k in range(2):
                            tp = ptr.tile([P, P], BF16, tag="trp")
                            nc.tensor.transpose(tp[:], ut[:, t, k * P:(k + 1) * P],
                                                ident[:])
                            nc.vector.tensor_copy(uT[k][:, t * P:(t + 1) * P],
                                                  tp[:])
                    zsb = io.tile([P, 2, 512], BF16, tag="zsb")
                    for m in range(2):
                        zp = pz.tile([P, 512], F32, tag="zp")
                        for k in range(2):
                            nc.tensor.matmul(zp[:], wg_sb[:, l * 4 + k * 2 + m, :],
                                             uT[k][:], start=(k == 0),
                                             stop=(k == 1))
                        nc.scalar.activation(zsb[:, m, :], zp[:], AF.Identity,
                                             accum_out=ssum[:, g, m:m + 1])
                        sq = io.tile([P, 512], F32, tag="sq")
                        nc.scalar.activation(sq[:], zp[:], AF.Square,
                                             accum_out=ssq[:, g, m:m + 1])
                    nc.sync.dma_start(
                        out=zt[l][:, c0:c0 + 512].rearrange(
                            "(m p) n -> p m n", p=P),
                        in_=zsb[:])

                # ---- stats allreduce + norm coefficients
                stl = dio.tile([P, 4], F32, tag="stl")
                nc.vector.tensor_reduce(stl[:, 0:1], ssum[:, :, 0],
                                        mybir.AxisListType.X, ALU.add)
                nc.vector.tensor_reduce(stl[:, 1:2], ssum[:, :, 1],
                                        mybir.AxisListType.X, ALU.add)
                nc.vector.tensor_reduce(stl[:, 2:3], ssq[:, :, 0],
                                        mybir.AxisListType.X, ALU.add)
                nc.vector.tensor_reduce(stl[:, 3:4], ssq[:, :, 1],
                                        mybir.AxisListType.X, ALU.add)
                nc.sync.dma_start(out=st_in[l][:, :], in_=stl[:])
                nc.gpsimd.collective_compute(
                    "AllReduce", ALU.add, replica_groups=rg,
                    ins=[st_in[l].ap().opt()], outs=[st_out[l].ap().opt()])
                stg = dio.tile([P, 4], F32, tag="stg")
                nc.sync.dma_start(out=stg[:], in_=st_out[l][:, :])
                mu = dio.tile([P, 2], F32, tag="mu")
                va = dio.tile([P, 2], F32, tag="va")
                aco = dio.tile([P, 2], F32, tag="aco")
                cco = dio.tile([P, 2], F32, tag="cco")
                inv_n = 1.0 / float(n_nodes)
                nc.vector.tensor_scalar(mu[:], stg[:, 0:2], inv_n, None, ALU.mult)
                nc.vector.tensor_scalar(va[:], stg[:, 2:4], inv_n, None, ALU.mult)
                mu2 = dio.tile([P, 2], F32, tag="mu2")
                nc.vector.tensor_tensor(mu2[:], mu[:], mu[:], ALU.mult)
                nc.vector.tensor_tensor(va[:], va[:], mu2[:], ALU.subtract)
                # rstd = 1/sqrt(va+eps)
                nc.scalar.activation(va[:], va[:], AF.Sqrt, bias=eps_sb[:, 0:1])
                nc.vector.reciprocal(va[:], va[:])
                for m in range(2):
                    nc.vector.tensor_tensor(aco[:, m:m + 1], gam[l][m],
                                            va[:, m:m + 1], ALU.mult)
                    nc.vector.tensor_tensor(cco[:, m:m + 1], mu[:, m:m + 1],
                                            aco[:, m:m + 1], ALU.mult)
                    nc.vector.tensor_tensor(cco[:, m:m + 1], bet[l][m],
                                            cco[:, m:m + 1], ALU.subtract)

                # ---- dense B: h = relu(a*z + c) [+ transpose or pool-accum]
                if l < L - 1:
                    for g in range(G):
                        c0 = g * 512
                        hrm = dio.tile([P, 4, H], BF16, tag="hrm")
                        zin = io.tile([P, 2, 512], BF16, tag="zin")
                        nc.sync.dma_start(
                            out=zin[:],
                            in_=zt[l][:, c0:c0 + 512].rearrange(
                                "(m p) n -> p m n", p=P))
                        for m in range(2):
                            hsb = io.tile([P, 512], BF16, tag="hsb")
                            nc.scalar.activation(hsb[:], zin[:, m, :], AF.Relu,
                                                 bias=cco[:, m:m + 1],
                                                 scale=aco[:, m:m + 1])
                            if g == G - 1 and LASTV < 512:
                                nc.vector.memset(hsb[:, LASTV:], 0.0)
                            transpose_to_rm(hsb, m, hrm)
                        nc.sync.dma_start(
                            out=hin[l + 1][c0:c0 + 512, :].rearrange(
                                "(t p) d -> p t d", p=P),
                            in_=hrm[:])
                else:
                    gsum = dio.tile([P, G, 2], F32, tag="gsum")
                    for g in range(G):
                        c0 = g * 512
                        nv = LASTV if g == G - 1 else 512
                        zin = io.tile([P, 2, 512], BF16, tag="zin")
                        nc.sync.dma_start(
                            out=zin[:],
                            in_=zt[l][:, c0:c0 + 512].rearrange(
                                "(m p) n -> p m n", p=P))
                        for m in range(2):
                            hsb = io.tile([P, 512], BF16, tag="hsb")
                            nc.scalar.activation(hsb[:, :nv], zin[:, m, :nv],
                                                 AF.Relu,
                                                 bias=cco[:, m:m + 1],
                                                 scale=aco[:, m:m + 1],
                                                 accum_out=gsum[:, g, m:m + 1])

            # ============================================================
            # mean pool allreduce + head
            gl = dio.tile([P, 2], F32, tag="gl")
            nc.vector.tensor_reduce(gl[:, 0:1], gsum[:, :, 0],
                                    mybir.AxisListType.X, ALU.add)
            nc.vector.tensor_reduce(gl[:, 1:2], gsum[:, :, 1],
                                    mybir.AxisListType.X, ALU.add)
            nc.sync.dma_start(out=g_in[:, :], in_=gl[:])
            nc.gpsimd.collective_compute(
                "AllReduce", ALU.add, replica_groups=rg,
                ins=[g_in.ap().opt()], outs=[g_out.ap().opt()])
            gg = dio.tile([P, 2], F32, tag="gg")
            nc.sync.dma_start(out=gg[:], in_=g_out[:, :])
            gbf = dio.tile([P, 2], BF16, tag="gbf")
            nc.vector.tensor_scalar(gbf[:], gg[:], 1.0 / float(n_nodes), None,
                                    ALU.mult)
            gp = dio.tile([P, 2], BF16, tag="gp")
            for m in range(2):
                pp = phead.tile([P, 1], F32, tag="pp")
                for k in range(2):
                    nc.tensor.matmul(pp[:], wp_sb[:, k * 2 + m, :],
                                     gbf[:, k:k + 1], start=(k == 0),
                                     stop=(k == 1))
                nc.scalar.activation(gp[:, m:m + 1], pp[:], AF.Relu,
                                     bias=b_pool[m])
            ph = phead.tile([T_HEADS, 1], F32, tag="ph")
            for k in range(2):
                nc.tensor.matmul(ph[:], wh_sb[:, k, :], gp[:, k:k + 1],
                                 start=(k == 0), stop=(k == 1))
            osb = dio.tile([T_HEADS, 1], F32, tag="osb")
            nc.scalar.activation(osb[:], ph[:], AF.Sigmoid,
                                 bias=b_head)
            nc.sync.dma_start(out=out_e[0:T_HEADS], in_=osb[:, 0])

    nc.compile()
    return nc


# ---------------------------------------------------------------- kernel

def _bf16(x):
    return np.asarray(x, np.float32).astype(ml_dtypes.bfloat16)


def build_in_maps(inputs, gidx_all, sidx_all, npc, padn):
    """Assemble per-core input maps from full inputs + prepped edge indices."""
    x = np.asarray(inputs["x"], np.float32)
    n_nodes = x.shape[0]
    ei = np.asarray(inputs["edge_index"])
    row = ei[0].astype(np.int64)
    b_in, W_in = inputs["b_in"], inputs["W_in"]
    W_gnn, gamma, beta = inputs["W_gnn"], inputs["gamma"], inputs["beta"]
    W_pool, b_pool = inputs["W_pool"], inputs["b_pool"]
    W_heads, b_heads = inputs["W_heads"], inputs["b_heads"]

    deg = np.bincount(row, minlength=n_nodes).astype(np.float32)
    rdeg = 1.0 / np.maximum(deg, 1.0)

    NCOEF = 2 + L * 2 + L * 2 + 2 + 1
    coef = np.zeros((P, NCOEF), np.float32)
    for m in range(2):
        coef[:, m] = b_in[m * P:(m + 1) * P]
        for l in range(L):
            coef[:, 2 + l * 2 + m] = gamma[l][m * P:(m + 1) * P]
            coef[:, 2 + 2 * L + l * 2 + m] = beta[l][m * P:(m + 1) * P]
        coef[:, 2 + 4 * L + m] = b_pool[m * P:(m + 1) * P]
    coef[:T_HEADS, 2 + 4 * L + 2] = b_heads

    winT = _bf16(np.asarray(W_in, np.float32).T)               # [F_IN, H]
    wgT = _bf16(np.asarray(W_gnn, np.float32).transpose(0, 2, 1))  # [L,H,H]
    wpT = _bf16(np.asarray(W_pool, np.float32).T)
    whT = _bf16(np.asarray(W_heads, np.float32).T)             # [H, T]

    in_maps = []
    for c in range(C):
        xs = np.zeros((F_IN, padn), ml_dtypes.bfloat16)
        xs[:, :npc] = _bf16(x[c * npc:(c + 1) * npc].T)
        rs = np.zeros(padn, np.float32)
        rs[:npc] = rdeg[c * npc:(c + 1) * npc]
        in_maps.append({
            "xT": xs, "rdeg": rs,
            "gsidx": gidx_all[c],
            "winT": winT, "wgT": wgT, "wpT": wpT, "whT": whT,
            "coef": coef,
        })
    return in_maps


def run_gnn(x, edge_index, W_in, b_in, W_gnn, b_gnn, gamma, beta,
            W_pool, b_pool, W_heads, b_heads, npc, trace=False):
    n_nodes = x.shape[0]
    assert n_nodes == npc * C
    padn = _round128(npc)
    if (padn // 512) * 512 != padn:
        padn = ((npc + 511) // 512) * 512

    ei = np.asarray(edge_index)
    gidx_all, sidx_all, meta = prep_edges(ei, n_nodes, npc, padn)
    inputs = dict(x=x, edge_index=edge_index, W_in=W_in, b_in=b_in,
                  W_gnn=W_gnn, gamma=gamma, beta=beta, W_pool=W_pool,
                  b_pool=b_pool, W_heads=W_heads, b_heads=b_heads)
    in_maps = build_in_maps(inputs, gidx_all, sidx_all, npc, padn)

    nc = build(meta, npc, padn, n_nodes)
    res = run_bass_kernel_spmd(nc, in_maps, core_ids=list(range(C)),
                               trace=trace)
    out = np.asarray(res.results[0]["out"], np.float32)
    return out, res


def kernel(**inputs) -> np.ndarray:
    out, _ = run_gnn(npc=25000, **inputs)
    return out
